# revision 41
# baseline (speedup 1.0000x reference)
"""TRN2 Bass kernel for nn_ChatDecoder: 4-layer causal decoder, 8 NeuronCores.

Sharding: data-parallel over batch (2 groups of 4 cores) x block-cyclic
sequence-parallel within each group. Core slot s owns global 128-token blocks
{4j+s : j=0..3} (512 tokens). The block-cyclic layout makes causal savings
uniform across cores: local q-block b only attends key blocks (r, c') with
c' <= b, 62.5% of the full score matrix, with masks needed only on the four
diagonal chunks (static per-core [128,128] tiles).

Per layer ONE AllGather moves K^T and v' (v interleaved with a ones column
per head for softmax denominators) in fp8e4 - half the wire bytes of bf16.
Matmuls keep bf16 moving operands (PE cost is keyed on the moving dtype);
fp8 is only the stationary K/V storage. Weights stay bf16.
"""
import sys
for _p in ("/opt/trn_rl_repo",):
    if _p not in sys.path:
        sys.path.insert(0, _p)


from contextlib import ExitStack

import numpy as np

SIM_NO_CC = False

import concourse.bass as bass
import concourse.mybir as mybir
import concourse.tile as tile
from concourse.masks import make_identity

FP32 = mybir.dt.float32
BF16 = mybir.dt.bfloat16
FP8 = mybir.dt.float8e4
AF = mybir.ActivationFunctionType
P = 128


def build_decoder(
    *,
    group_size: int,
    tpc: int,
    layers: int,
    d_model: int,
    n_heads: int,
    ffn: int,
    vocab: int,
    attn_scale: float,
    replica_groups,
    eps: float = 1e-5,
    ln_gamma_beta: bool = False,
    qkv_bias: bool = False,
    proj_bias: bool = False,
    f1_bias: bool = False,
    f2_bias: bool = False,
    out_x_only: bool = False,
    mm_nfree: int = 512,
    kv_fp8: bool = True,
    reps: int = 1,
):
    TM = tpc // P                 # local 128-token blocks (4)
    DK = d_model // P             # feature blocks (8)
    HP = n_heads // 2             # head pairs (8)
    HD = d_model // n_heads       # head dim (64)
    FFK = ffn // P
    VH = HD + 1                   # v' head slice: 64 v cols + ones col
    KW = DK * tpc                 # kown slab width (4096)
    VW = TM * n_heads * VH        # vown slab width (4*1040=4160)
    KVW = KW + VW                 # combined slab width (8256)
    R = group_size
    KVD = FP8 if kv_fp8 else BF16
    assert HD == 64 and P == 128 and TM == R

    nc = bass.Bass()

    x0_d = nc.dram_tensor("x0", [tpc, d_model], FP32, kind="ExternalInput")
    # mask[r]: [key off, query off] for diagonal chunks; ones / tri / zeros
    mask_d = nc.dram_tensor("maskT", [R, P, P], BF16, kind="ExternalInput")
    qkv_wT_d = nc.dram_tensor("qkv_wT", [layers, d_model, 3 * d_model], BF16, kind="ExternalInput")
    proj_wT_d = nc.dram_tensor("proj_wT", [layers, d_model, d_model], BF16, kind="ExternalInput")
    f1_wT_d = nc.dram_tensor("f1_wT", [layers, d_model, ffn], BF16, kind="ExternalInput")
    f2_wT_d = nc.dram_tensor("f2_wT", [layers, ffn, d_model], BF16, kind="ExternalInput")
    if ln_gamma_beta:
        ln_wb_d = nc.dram_tensor("ln_wb", [2 * layers + 1, 2, d_model], FP32, kind="ExternalInput")
    qkv_b_d = nc.dram_tensor("qkv_b", [layers, 3 * d_model], FP32, kind="ExternalInput") if qkv_bias else None
    proj_b_d = nc.dram_tensor("proj_b", [layers, d_model], FP32, kind="ExternalInput") if proj_bias else None
    f1_b_d = nc.dram_tensor("f1_b", [layers, ffn], FP32, kind="ExternalInput") if f1_bias else None
    f2_b_d = nc.dram_tensor("f2_b", [layers, d_model], FP32, kind="ExternalInput") if f2_bias else None
    if out_x_only:
        out_d = nc.dram_tensor("out", [tpc, d_model], FP32, kind="ExternalOutput")
        lm_wT_d = None
    else:
        lm_wT_d = nc.dram_tensor("lm_wT", [d_model, vocab], BF16, kind="ExternalInput")
        out_d = nc.dram_tensor("out", [tpc, vocab], FP32, kind="ExternalOutput")

    with tile.TileContext(nc) as tc, ExitStack() as ctx:
        # ---------- persistent pools ----------
        const = ctx.enter_context(tc.tile_pool(name="const", bufs=1))
        ident = const.tile([P, P], BF16)
        make_identity(nc, ident)
        eps_t = const.tile([P, 1], FP32)
        nc.vector.memset(eps_t, eps)
        ones_t = const.tile([P, P], BF16, tag="ones", name="ones")
        nc.vector.memset(ones_t, 1.0)

        xp = ctx.enter_context(tc.tile_pool(name="x", bufs=1))
        x = [xp.tile([P, d_model], FP32, tag=f"x{m}", name=f"x{m}") for m in range(TM)]

        mkp = ctx.enter_context(tc.tile_pool(name="mask", bufs=1))
        maskT = [mkp.tile([P, P], BF16, tag=f"mask{r}", name=f"mask{r}") for r in range(R)]
        for r in range(R):
            nc.sync.dma_start(maskT[r], mask_d[r])

        if ln_gamma_beta:
            lnp = ctx.enter_context(tc.tile_pool(name="lnwb", bufs=1))
            ln_wb = []
            for i in range(2 * layers + 1):
                g = lnp.tile([P, d_model], FP32, tag=f"g{i}", name=f"g{i}")
                b = lnp.tile([P, d_model], FP32, tag=f"b{i}", name=f"b{i}")
                for dst, row in ((g, ln_wb_d[i, 0, :]), (b, ln_wb_d[i, 1, :])):
                    nc.sync.dma_start(dst, bass.AP(tensor=row.tensor, offset=row.offset,
                                                   ap=[[0, P]] + list(row.ap)))
                ln_wb.append((g, b))

        sp = ctx.enter_context(tc.tile_pool(name="sp", bufs=4))       # LN stats etc
        fmp = ctx.enter_context(tc.tile_pool(name="fm", bufs=1))      # xn/xnT/qT/yT
        kvp = ctx.enter_context(tc.tile_pool(name="kv", bufs=1))      # K/V slabs
        dramp = ctx.enter_context(tc.tile_pool(name="dram", bufs=2, space="DRAM"))

        # K/V slabs. kown: [128 feat-in-block, (dk, tok)]; vown: [128 tok-in-block,
        # (c', head, vh)]. Gathered versions hold all R ranks side by side.
        kown = kvp.tile([P, KW], KVD, tag="kown", name="kown")
        vown = kvp.tile([P, VW], KVD, tag="vown", name="vown")
        kt_sb = kvp.tile([P, R * KW], KVD, tag="ktsb", name="ktsb")
        v_sb = kvp.tile([P, R * VW], KVD, tag="vsb", name="vsb")
        # static ones columns of v' (e == HD within each head slice)
        nc.vector.memset(
            vown.rearrange("p (c j e) -> p c j e", j=n_heads, e=VH)[:, :, :, HD:VH], 1.0)

        # ---------- helpers ----------
        def layer_norm(idx, dst_tag, ms=None):
            xn = []
            for m in (range(TM) if ms is None else ms):
                st = sp.tile([P, d_model // 512, 6], FP32)
                mv = sp.tile([P, 2], FP32)
                for sg in range(d_model // 512):
                    nc.vector.bn_stats(st[:, sg, :], x[m][:, sg * 512:(sg + 1) * 512])
                nc.vector.bn_aggr(mv, st)
                nc.scalar.activation(mv[:, 1:2], mv[:, 1:2], AF.Sqrt, bias=eps_t, scale=1.0)
                nc.vector.reciprocal(mv[:, 1:2], mv[:, 1:2])
                t = fmp.tile([P, d_model], BF16, tag=f"{dst_tag}{m}", name=f"{dst_tag}{m}")
                if ln_gamma_beta:
                    tf = sp.tile([P, d_model], FP32, tag=f"lnf{m}", name=f"lnf{m}")
                    nc.vector.tensor_scalar(tf, x[m], scalar1=mv[:, 0:1], scalar2=mv[:, 1:2],
                                            op0=mybir.AluOpType.subtract, op1=mybir.AluOpType.mult)
                    g, b = ln_wb[idx]
                    nc.vector.tensor_mul(tf, tf, g)
                    nc.vector.tensor_add(t, tf, b)
                else:
                    nc.vector.tensor_scalar(t, x[m], scalar1=mv[:, 0:1], scalar2=mv[:, 1:2],
                                            op0=mybir.AluOpType.subtract, op1=mybir.AluOpType.mult)
                xn.append(t)
            return xn

        def transpose_tm_to_fm(src_tiles, n_feat, dst_tag, pool):
            out = []
            with tc.tile_pool(name="ps_t", bufs=3, space="PSUM") as ps_t:
                for dk in range(n_feat // P):
                    pt = ps_t.tile([P, tpc], BF16, tag="pt", name=f"pt{dk}")
                    for m in range(TM):
                        nc.tensor.transpose(pt[:, m * P:(m + 1) * P],
                                            src_tiles[m][:, dk * P:(dk + 1) * P], ident)
                    t = pool.tile([P, tpc], BF16, tag=f"{dst_tag}{dk}", name=f"{dst_tag}{dk}")
                    nc.vector.tensor_copy(t, pt)
                    out.append(t)
            return out

        def mm_weight_stationary(w_dram, actT, n_out, dst_tag, pool, evict, b_dram=None,
                                 ng_order=None, tok0=0, tok1=None):
            """evict(M, ps, bt) consumes PSUM tile ps for output block M.
            tok0/tok1 restrict to a token-column range of actT."""
            t1 = tpc if tok1 is None else tok1
            tw = t1 - tok0
            with tc.tile_pool(name="wp", bufs=3) as wp, \
                 tc.tile_pool(name="ps_m", bufs=4, space="PSUM") as ps_m:
                wv = w_dram.rearrange("(k p) n -> p k n", p=P)
                for ng in (ng_order or range(n_out // 512)):
                    wt = wp.tile([P, DK * 512], BF16, tag="w", name=f"w{ng}")
                    nc.sync.dma_start(wt.rearrange("p (k n) -> p k n", n=512),
                                      wv[:, :, ng * 512:(ng + 1) * 512])
                    for j in range(4):
                        M = ng * 4 + j
                        ps = ps_m.tile([P, tpc], FP32, tag="ps", name=f"ps{M}_{tok0}")
                        for k in range(DK):
                            nc.tensor.matmul(ps[:, tok0:t1],
                                             lhsT=wt[:, k * 512 + j * P:k * 512 + (j + 1) * P],
                                             rhs=actT[k][:, tok0:t1],
                                             start=(k == 0), stop=(k == DK - 1))
                        bt = None
                        if b_dram is not None:
                            bt = sp.tile([P, 1], FP32, tag="bias", name=f"bias{M}")
                            nc.sync.dma_start(bt, b_dram[M * P:(M + 1) * P].rearrange("a -> a 1"))
                        evict(M, ps, bt, tok0, t1)

        def mm_act_stationary_residual(w_dram, actT, nk, wbufs=2, nfree=512, b_dram=None):
            nbanks = (nfree * 4 + 2047) // 2048
            bb = None
            if b_dram is not None:
                bb = sp.tile([P, d_model], FP32, tag="biasrow", name="biasrow")
                nc.sync.dma_start(bb, bass.AP(tensor=b_dram.tensor, offset=b_dram.offset,
                                              ap=[[0, P]] + list(b_dram.ap)))
            KC = min(nk, 8)  # k-tiles per weight-load chunk (pipeline DMA/compute)
            with tc.tile_pool(name="wpa", bufs=2) as wp, \
                 tc.tile_pool(name="ps_m", bufs=2, space="PSUM") as ps_m:
                wv = w_dram.rearrange("(k p) n -> p k n", p=P)
                for n in range(d_model // nfree):
                    pss = [ps_m.tile([P, nfree], FP32, tag=f"ps{m}", name=f"ps{n}_{m}")
                           for m in range(TM)]
                    for kc in range(nk // KC):
                        wt = wp.tile([P, KC * nfree], BF16, tag="w", name=f"w{n}_{kc}")
                        nc.sync.dma_start(wt.rearrange("p (k n) -> p k n", n=nfree),
                                          wv[:, kc * KC:(kc + 1) * KC,
                                             n * nfree:(n + 1) * nfree])
                        for m in range(TM):
                            for k2 in range(KC):
                                k = kc * KC + k2
                                nc.tensor.matmul(pss[m], lhsT=actT[k][:, m * P:(m + 1) * P],
                                                 rhs=wt[:, k2 * nfree:(k2 + 1) * nfree],
                                                 start=(k == 0), stop=(k == nk - 1))
                    for m in range(TM):
                        ps = pss[m]
                        xs = x[m][:, n * nfree:(n + 1) * nfree]
                        if bb is not None:
                            nc.vector.tensor_add(ps, ps, bb[:, n * nfree:(n + 1) * nfree])
                        nc.vector.tensor_add(xs, ps, xs)

        for rep in range(reps):
            for m in range(TM):
                nc.sync.dma_start(x[m], x0_d[m * P:(m + 1) * P, :])
            # ---------- layers ----------
            for l in range(layers):
                # --- per token half: LN -> xnT -> K,V -> gather, so gather A
                # launches ~25us into the layer and B right behind it
                qT = [fmp.tile([P, tpc], BF16, tag=f"qT{dk}", name=f"qT{dk}_{l}")
                      for dk in range(DK)]
                vT_own = [fmp.tile([P, tpc], BF16, tag=f"vT{dk}", name=f"vT{dk}_{l}")
                          for dk in range(DK)]
                xnT = [fmp.tile([P, tpc], BF16, tag=f"xnT{dk}", name=f"xnT{dk}_{l}")
                       for dk in range(DK)]

                def qkv_evict(M, ps, bt, t0, t1):
                    dk = M % DK
                    kind = M // DK      # 0=Q 1=K 2=V
                    if bt is not None:
                        nc.vector.tensor_scalar(ps[:, t0:t1], ps[:, t0:t1], scalar1=bt,
                                                scalar2=None, op0=mybir.AluOpType.add)
                    if kind == 1:
                        nc.vector.tensor_copy(kown[:, dk * tpc + t0:dk * tpc + t1],
                                              ps[:, t0:t1])
                    elif kind == 0:
                        nc.vector.tensor_copy(qT[dk][:, t0:t1], ps[:, t0:t1])
                    else:
                        nc.vector.tensor_copy(vT_own[dk][:, t0:t1], ps[:, t0:t1])

                HT = tpc // 2            # token half (256)
                HB = TM // 2             # blocks per half (2)
                KW2 = DK * HT            # K bytes-per-part per half slab (2048)
                VW2 = HB * n_heads * VH  # v' width per half (2080)
                gkv_h = []
                for h in range(2):
                    t0, t1 = h * HT, (h + 1) * HT
                    ms = (h * HB, h * HB + 1)
                    xnh = layer_norm(2 * l, "xn", ms=ms)
                    with tc.tile_pool(name="ps_tx", bufs=3, space="PSUM") as ps_tx:
                        for dk in range(DK):
                            pt = ps_tx.tile([P, HT], BF16, tag="ptx", name=f"ptx{dk}_{h}")
                            for i, m in enumerate(ms):
                                nc.tensor.transpose(pt[:, i * P:(i + 1) * P],
                                                    xnh[i][:, dk * P:(dk + 1) * P], ident)
                            nc.vector.tensor_copy(xnT[dk][:, t0:t1], pt)
                    mm_weight_stationary(qkv_wT_d[l], xnT, 3 * d_model, "qkvT", None,
                                         qkv_evict,
                                         b_dram=qkv_b_d[l] if qkv_bias else None,
                                         ng_order=[2, 3, 4, 5], tok0=t0, tok1=t1)
                    # v feature-major -> token-major, interleave (ones cols static)
                    with tc.tile_pool(name="ps_t", bufs=3, space="PSUM") as ps_t:
                        for m in (h * HB, h * HB + 1):
                            pt = ps_t.tile([P, d_model], BF16, tag="pt", name=f"ptv{m}")
                            for dk in range(DK):
                                nc.tensor.transpose(pt[:, dk * P:(dk + 1) * P],
                                                    vT_own[dk][:, m * P:(m + 1) * P], ident)
                            vb = sp.tile([P, d_model], BF16, tag="vbounce", name=f"vb{m}")
                            nc.vector.tensor_copy(vb, pt)
                            dst = vown[:, m * n_heads * VH:(m + 1) * n_heads * VH]
                            nc.gpsimd.dma_start(
                                dst.rearrange("p (j e) -> p j e", e=VH)[:, :, 0:HD],
                                vb.rearrange("p (j e) -> p j e", e=HD))
                    kv_in = dramp.tile([P, KW2 + VW2], KVD, tag=f"kv_in{h}", name=f"kv_in{h}")
                    nc.sync.dma_start(kv_in[:, 0:KW2].rearrange("p (k n) -> p k n", n=HT),
                                      kown.rearrange("p (k n) -> p k n", n=tpc)[:, :, t0:t1])
                    nc.sync.dma_start(kv_in[:, KW2:], vown[:, h * VW2:(h + 1) * VW2])
                    gkv = dramp.tile([R, P, KW2 + VW2], KVD, tag=f"gkv{h}", name=f"gkv{h}")
                    gkv_h.append(gkv)
                    if SIM_NO_CC:
                        nc.sync.dma_start(gkv[0], kv_in)
                    else:
                        nc.gpsimd.collective_compute("AllGather", mybir.AluOpType.bypass,
                                                     replica_groups=replica_groups,
                                                     ins=[kv_in.opt()], outs=[gkv.opt()])
                # Q blocks while the gathers are in flight
                mm_weight_stationary(qkv_wT_d[l], xnT, 3 * d_model, "qkvT", None,
                                     qkv_evict,
                                     b_dram=qkv_b_d[l] if qkv_bias else None,
                                     ng_order=[0, 1])

                # assembly: half h token range [h*HT, (h+1)*HT) per rank
                for h in range(2):
                    t0 = h * HT
                    for r in range(R):
                        eng = nc.gpsimd
                        dst = kt_sb[:, r * KW:(r + 1) * KW]
                        eng.dma_start(
                            dst.rearrange("p (k n) -> p k n", n=tpc)[:, :, t0:t0 + HT],
                            gkv_h[h][r, :, 0:KW2].rearrange("p (k n) -> p k n", n=HT))
                        eng.dma_start(
                            v_sb[:, r * VW + h * VW2:r * VW + (h + 1) * VW2],
                            gkv_h[h][r, :, KW2:])

                # --- attention, block-cyclic causal, two phases ---
                # chunk (r, c') covers global key block 4c'+r; q-block b covers
                # global 4b+s. Compute chunks c'<=b; diagonal (c'==b) masked by
                # maskT[r] (ones r<s / tri r==s / zeros r>s).
                # Phase 1 processes c' in {0,1} (only needs gather A) across ALL
                # head pairs, filling gather B's flight; phase 2 does c' in
                # {2,3}. Per-phase pv partials accumulate into SBUF fp32 (yacc)
                # to dodge the 8-bank PSUM limit.
                yT = []
                yacc = [(fmp.tile([P, tpc], FP32, tag=f"yaccA{hp}", name=f"yaccA{hp}"),
                         fmp.tile([P, tpc], FP32, tag=f"yaccB{hp}", name=f"yaccB{hp}"))
                        for hp in range(HP)]
                with tc.tile_pool(name="pp", bufs=3) as pp, \
                     tc.tile_pool(name="rp", bufs=2) as rp, \
                     tc.tile_pool(name="ps_s", bufs=2, space="PSUM") as ps_s, \
                     tc.tile_pool(name="ps_y", bufs=1, space="PSUM") as ps_y:
                    def attn_phase(hp, cs, bs):
                        """scores+exp+mask for chunks c in cs, then pv for
                        q-blocks bs (complete w.r.t. cs); returns psum pair."""
                        pAB = {}
                        for c in cs:
                            w = (TM - c) * P          # q suffix width
                            q0 = c * P
                            for r in range(R):
                                col = r * KW + hp * tpc + c * P
                                ps2 = ps_s.tile([P, 2 * tpc], FP32, tag="ps2",
                                                name=f"s{hp}_{r}_{c}")
                                nc.tensor.matmul(ps2[:, 0:w], lhsT=kt_sb[0:HD, col:col + P],
                                                 rhs=qT[hp][0:HD, q0:tpc], start=True, stop=True,
                                                 tile_position=(0, 0))
                                nc.tensor.matmul(ps2[:, tpc:tpc + w],
                                                 lhsT=kt_sb[HD:P, col:col + P],
                                                 rhs=qT[hp][HD:P, q0:tpc], start=True, stop=True,
                                                 tile_position=(64, 0))
                                tp2 = pp.tile([P, 2 * w], BF16, tag=f"p{r}_{c % 2}",
                                              name=f"p{hp}_{r}_{c}")
                                nc.scalar.activation(
                                    tp2.rearrange("p (two v) -> p two v", two=2),
                                    ps2.rearrange("p (two t) -> p two t", two=2)[:, :, 0:w],
                                    AF.Exp, scale=attn_scale)
                                nc.vector.tensor_mul(tp2[:, 0:P], tp2[:, 0:P], maskT[r])
                                nc.vector.tensor_mul(tp2[:, w:w + P], tp2[:, w:w + P], maskT[r])
                                pAB[(r, c)] = (tp2, w)
                        pyA = ps_y.tile([P, tpc], FP32, tag="pyA", name=f"pyA{hp}_{cs[0]}")
                        pyB = ps_y.tile([P, tpc], FP32, tag="pyB", name=f"pyB{hp}_{cs[0]}")
                        for b in bs:
                            ci = [c for c in cs if c <= b]
                            nchunks = R * len(ci)
                            i = 0
                            for c in ci:
                                for r in range(R):
                                    vA = v_sb[:, r * VW + c * n_heads * VH + (2 * hp) * VH:
                                              r * VW + c * n_heads * VH + (2 * hp + 1) * VH]
                                    vB = v_sb[:, r * VW + c * n_heads * VH + (2 * hp + 1) * VH:
                                              r * VW + c * n_heads * VH + (2 * hp + 2) * VH]
                                    qs = (b - c) * P
                                    tp2, w = pAB[(r, c)]
                                    nc.tensor.matmul(pyA[0:VH, b * P:(b + 1) * P], lhsT=vA,
                                                     rhs=tp2[:, qs:qs + P],
                                                     start=(i == 0), stop=(i == nchunks - 1),
                                                     skip_group_check=True)
                                    nc.tensor.matmul(pyB[0:VH, b * P:(b + 1) * P], lhsT=vB,
                                                     rhs=tp2[:, w + qs:w + qs + P],
                                                     start=(i == 0), stop=(i == nchunks - 1),
                                                     skip_group_check=True)
                                    i += 1
                        return pyA, pyB

                    for hp in range(HP):      # phase 1: chunks c' in {0,1}
                        pyA, pyB = attn_phase(hp, [0, 1], [0, 1, 2, 3])
                        aA, aB = yacc[hp]
                        nc.vector.tensor_copy(aA[0:VH, :], pyA[0:VH, :])
                        nc.vector.tensor_copy(aB[0:VH, :], pyB[0:VH, :])
                    for hp in range(HP):      # phase 2: chunks c' in {2,3}
                        pyA, pyB = attn_phase(hp, [2, 3], [2, 3])
                        aA, aB = yacc[hp]
                        nc.vector.tensor_add(aA[0:VH, 2 * P:], aA[0:VH, 2 * P:],
                                             pyA[0:VH, 2 * P:])
                        nc.vector.tensor_add(aB[0:VH, 2 * P:], aB[0:VH, 2 * P:],
                                             pyB[0:VH, 2 * P:])
                    for hp in range(HP):      # epilogue: normalize
                        aA, aB = yacc[hp]
                        # denominators: row HD -> reciprocal -> broadcast via
                        # rank-1 matmul (ones column x denom row)
                        dn = rp.tile([P, 2 * tpc], BF16, tag="dn", name=f"dn{hp}")
                        with nc.allow_low_precision(reason="softmax denom bf16 broadcast"):
                            nc.vector.reciprocal(dn[HD:HD + 1, 0:tpc], aA[HD:HD + 1, :])
                            nc.vector.reciprocal(dn[HD:HD + 1, tpc:2 * tpc], aB[HD:HD + 1, :])
                        rbA = ps_y.tile([P, tpc], FP32, tag="rbA", name=f"rbA{hp}")
                        rbB = ps_y.tile([P, tpc], FP32, tag="rbB", name=f"rbB{hp}")
                        nc.tensor.matmul(rbA[0:HD, :], lhsT=ones_t[HD:HD + 1, 0:HD],
                                         rhs=dn[HD:HD + 1, 0:tpc], start=True, stop=True,
                                         tile_position=(64, 0))
                        nc.tensor.matmul(rbB[0:HD, :], lhsT=ones_t[HD:HD + 1, 0:HD],
                                         rhs=dn[HD:HD + 1, tpc:2 * tpc], start=True, stop=True,
                                         tile_position=(64, 0))
                        rsA = rp.tile([P, tpc], FP32, tag="rsA", name=f"rsA{hp}")
                        nc.vector.tensor_copy(rsA[0:HD, :], rbA[0:HD, :])
                        rsB = rp.tile([P, tpc], FP32, tag="rsB", name=f"rsB{hp}")
                        nc.vector.tensor_copy(rsB[0:HD, :], rbB[0:HD, :])
                        t = fmp.tile([P, tpc], BF16, tag=f"yT{hp}", name=f"yT{hp}")
                        tmpB = rp.tile([P, tpc], BF16, tag="tmpB", name=f"tmpB{hp}")
                        nc.vector.tensor_mul(t[0:HD, :], aA[0:HD, :], rsA[0:HD, :])
                        nc.vector.tensor_mul(tmpB[0:HD, :], aB[0:HD, :], rsB[0:HD, :])
                        nc.gpsimd.dma_start(t[HD:P, :], tmpB[0:HD, :])
                        yT.append(t)

                # proj + residual
                mm_act_stationary_residual(proj_wT_d[l], yT, DK, nfree=mm_nfree,
                                           b_dram=proj_b_d[l] if proj_bias else None)

                # LN2 + FFN
                xn2 = layer_norm(2 * l + 1, "xn")
                xn2T = transpose_tm_to_fm(xn2, d_model, "xnT", fmp)

                h4T = [None] * FFK

                with tc.tile_pool(name="h4p", bufs=1) as h4p:
                    def gelu_evict(M, ps, bt, t0=0, t1=tpc):
                        t = h4p.tile([P, tpc], BF16, tag=f"h4T{M}", name=f"h4T{M}")
                        if bt is None:
                            nc.scalar.activation(t, ps, AF.Gelu)
                        else:
                            nc.scalar.activation(t, ps, AF.Gelu, bias=bt)
                        h4T[M] = t

                    mm_weight_stationary(f1_wT_d[l], xn2T, ffn, "h4T", None,
                                         gelu_evict,
                                         b_dram=f1_b_d[l] if f1_bias else None)
                    mm_act_stationary_residual(f2_wT_d[l], h4T, FFK, wbufs=1, nfree=mm_nfree,
                                               b_dram=f2_b_d[l] if f2_bias else None)

            # ---------- final LN + LM head ----------
            xf = layer_norm(2 * layers, "xn")
            if out_x_only:
                for m in range(TM):
                    o = sp.tile([P, d_model], FP32, tag=f"xo{m}", name=f"xo{m}")
                    nc.vector.tensor_copy(o, xf[m])
                    nc.sync.dma_start(out_d[m * P:(m + 1) * P, :], o)
            else:
                xfT = transpose_tm_to_fm(xf, d_model, "xnT", fmp)
                NF = mm_nfree
                nvt = (vocab + NF - 1) // NF
                nbanks = (NF * 4 + 2047) // 2048
                with tc.tile_pool(name="wp", bufs=3) as wp, \
                     tc.tile_pool(name="lop", bufs=3) as lop, \
                     tc.tile_pool(name="ps_m", bufs=max(2, 8 // nbanks), space="PSUM") as ps_m:
                    lwv = lm_wT_d.rearrange("(k p) n -> p k n", p=P)
                    for nt in range(nvt):
                        nsz = min(NF, vocab - nt * NF)
                        wt = wp.tile([P, DK * NF], BF16, tag="w", name=f"w{nt}")
                        nc.sync.dma_start(
                            wt.rearrange("p (k n) -> p k n", n=NF)[:, :, 0:nsz],
                            lwv[:, :, nt * NF:nt * NF + nsz])
                        for m in range(TM):
                            ps = ps_m.tile([P, NF], FP32, tag="ps", name=f"lps{nt}_{m}")
                            for k in range(DK):
                                nc.tensor.matmul(ps[:, 0:nsz], lhsT=xfT[k][:, m * P:(m + 1) * P],
                                                 rhs=wt[:, k * NF:k * NF + nsz],
                                                 start=(k == 0), stop=(k == DK - 1))
                            lt = lop.tile([P, NF], FP32, tag=f"lo{m % 2}", name=f"lo{nt}_{m}")
                            (nc.scalar.copy if m % 2 == 0 else nc.vector.tensor_copy)(
                                lt[:, 0:nsz], ps[:, 0:nsz])
                            nc.sync.dma_start(out_d[m * P:(m + 1) * P, nt * NF:nt * NF + nsz],
                                              lt[:, 0:nsz])

    return nc


# ======================================================================
# Host-side wrapper: full inputs in, full logits out.
# ======================================================================
import json as _json

import ml_dtypes as _mld

_BF = _mld.bfloat16

_L, _H, _D, _V = 4, 16, 1024, 32000
_B, _T = 2, 2048
_GROUP, _TPC = 4, 512
_N_CORES = 8


def _to_bf16(a):
    return np.ascontiguousarray(np.asarray(a, np.float32).astype(_BF))


def _make_masks(slot):
    """mask[r] [key off i, query off j] for diagonal chunks of slot s."""
    m = np.zeros((_GROUP, 128, 128), np.float32)
    tri = (np.arange(128)[:, None] <= np.arange(128)[None, :])
    for r in range(_GROUP):
        if r < slot:
            m[r] = 1.0
        elif r == slot:
            m[r] = tri
    return m.astype(_BF)


def _token_perm(slot):
    """global token indices owned by core slot, in local order."""
    idx = []
    for b in range(4):
        g0 = (4 * b + slot) * 128
        idx.extend(range(g0, g0 + 128))
    return np.array(idx)


def _split_excess_waits(bir: bytes) -> bytes:
    """This container's walrus accepts only one sync-wait per instruction;
    hoist extra waits onto preceding NoOps on the same engine."""
    m = _json.loads(bir)
    changed = False
    for fn in m["functions"]:
        for bb in fn["blocks"]:
            out = []
            for inst in bb["instructions"]:
                si = inst.get("sync_info")
                if si:
                    waits = si.get("on_wait") or []
                    if len(waits) > 1:
                        changed = True
                        for j, wt in enumerate(waits[:-1]):
                            out.append({
                                "debug": inst.get("debug", 0),
                                "engine": inst["engine"],
                                "ins": [],
                                "name": inst["name"] + f"_dw{j}",
                                "opcode": "NoOp",
                                "outs": [],
                                "sync_info": {"on_update": [], "on_wait": [wt]},
                            })
                        si["on_wait"] = [waits[-1]]
                out.append(inst)
            bb["instructions"] = out
    return _json.dumps(m).encode() if changed else bir


def kernel(token_ids, context_emb, tok_emb, pos_emb, qkv_w, qkv_b, proj_w, proj_b,
           ln1_w, ln1_b, ln2_w, ln2_b, f1_w, f1_b, f2_w, f2_b, lnf_w, lnf_b):
    from concourse.bass_utils import run_bass_kernel_spmd

    token_ids = np.asarray(token_ids)
    tok_emb = np.asarray(tok_emb, np.float32)
    pos_emb = np.asarray(pos_emb, np.float32)
    context_emb = np.asarray(context_emb, np.float32)
    qkv_b = np.asarray(qkv_b, np.float32)
    proj_b = np.asarray(proj_b, np.float32)
    f1_b = np.asarray(f1_b, np.float32)
    f2_b = np.asarray(f2_b, np.float32)

    # embedding on host (pure gather + add; all matmul FLOPs run on device)
    x0_full = tok_emb[token_ids] + pos_emb[:_T][None] + context_emb[:, None, :]
    x0_full = x0_full.astype(np.float32)

    ln_identity = (np.all(ln1_w == 1) and np.all(ln2_w == 1) and np.all(lnf_w == 1)
                   and not np.any(ln1_b) and not np.any(ln2_b) and not np.any(lnf_b))
    flags = dict(
        ln_gamma_beta=not ln_identity,
        qkv_bias=bool(np.any(qkv_b)),
        proj_bias=bool(np.any(proj_b)),
        f1_bias=bool(np.any(f1_b)),
        f2_bias=bool(np.any(f2_b)),
    )

    nc = build_decoder(group_size=_GROUP, tpc=_TPC, layers=_L, d_model=_D,
                       n_heads=_H, ffn=4 * _D, vocab=_V,
                       attn_scale=1.0 / float(np.sqrt(_D // _H)),
                       replica_groups=[[0, 1, 2, 3], [4, 5, 6, 7]], **flags)
    orig = nc.to_json_bytes
    nc.to_json_bytes = lambda: _split_excess_waits(orig())

    w = {
        "qkv_wT": _to_bf16(np.transpose(np.asarray(qkv_w, np.float32), (0, 2, 1))),
        "proj_wT": _to_bf16(np.transpose(np.asarray(proj_w, np.float32), (0, 2, 1))),
        "f1_wT": _to_bf16(np.transpose(np.asarray(f1_w, np.float32), (0, 2, 1))),
        "f2_wT": _to_bf16(np.transpose(np.asarray(f2_w, np.float32), (0, 2, 1))),
        "lm_wT": _to_bf16(tok_emb.T),
    }

    in_maps = []
    perms = [_token_perm(s) for s in range(_GROUP)]
    for c in range(_N_CORES):
        b, sl = divmod(c, _GROUP)
        im = {
            "x0": np.ascontiguousarray(x0_full[b, perms[sl], :], np.float32),
            "maskT": _make_masks(sl),
            **w,
        }
        if flags["ln_gamma_beta"]:
            wb = np.zeros((2 * _L + 1, 2, _D), np.float32)
            for l in range(_L):
                wb[2 * l, 0], wb[2 * l, 1] = ln1_w[l], ln1_b[l]
                wb[2 * l + 1, 0], wb[2 * l + 1, 1] = ln2_w[l], ln2_b[l]
            wb[2 * _L, 0], wb[2 * _L, 1] = lnf_w, lnf_b
            im["ln_wb"] = wb
        if flags["qkv_bias"]:
            im["qkv_b"] = qkv_b
        if flags["proj_bias"]:
            im["proj_b"] = proj_b
        if flags["f1_bias"]:
            im["f1_b"] = f1_b
        if flags["f2_bias"]:
            im["f2_b"] = f2_b
        in_maps.append(im)

    res = run_bass_kernel_spmd(nc, in_maps, core_ids=list(range(_N_CORES)))

    out = np.zeros((_B, _T, _V), np.float32)
    for c in range(_N_CORES):
        b, sl = divmod(c, _GROUP)
        out[b, perms[sl], :] = res.results[c]["out"]
    return out
